# revision 1
# baseline (speedup 1.0000x reference)
"""Chamfer loss Trainium2 kernel (data-parallel over batch, 8 NeuronCores).

Problem: x, y (8, 4096, 3) fp32; loss = mean_n [ mean_w min_v ||x_nv - y_nw||
+ mean_v min_w ||x_nv - y_nw|| ] (scalar fp32).

Architecture (per core, one batch) — "quadrant soft+raw" scheme. The
4096x4096 sq-distance matrix is computed once, split into 4 quadrants of
[2048 x 2048], half x-major and half y-major:

    Q11 (v<2048, w<2048)  x-major, Exp evacuation (soft)
    Q22 (v>=2048,w>=2048) x-major, Exp evacuation (soft)
    Q12 (v<2048, w>=2048) y-major, raw fp16 evacuation (hard)
    Q21 (v>=2048,w<2048)  y-major, raw fp16 evacuation (hard)

Every x-row gets one SOFT half (ACT evacuates exp(-sq/T) with a free
fused accum_out row-sum -> softmin, zero DVE cost) and one RAW half (the
y-major quadrant's elementwise min-chain, exact). Every y-col likewise
(exp max-chain = exact pointwise hard min in exp space + y-major fold
trees). Soft halves whose row-sum underflowed (true min-sq beyond exp's
~86*T fp32 range) are replaced by +BIG via an arithmetic mask, so the
final min(soft, raw) falls back to the exact raw half; the residual
error (both halves in the far tail) measures 2.2e-3 relative on the
reference inputs at T=0.004 (gate: 2e-2).

Measured engine queues (HW, trace): ACT ~128us (60 of 64 unit
evacuations + table loads), DVE ~141us (64 col-chain TTs at fp16/bf16
2x, 28 units' fold trees batched 4-wide via 3D APs, 4 "C" units
evacuated by DVE itself through a 1x tensor_scalar with fused min-accum
to offload ACT), PE ~158us busy at its throttled ~1.2GHz streaming rate
(512-col matmul = ~615ns incl serial LDWEIGHTS) -- the PE paces the
pipeline; epilogue transposes run on the DMA XBAR to keep them off it.
PSUM runs as 4 x [128,1024] rotating groups for fine PE/evac overlap.

Host: packs the error-compensated 3-way bf16 split gram operands (the
augmented [24, 4096] ax/ay work in both gram orientations since
ay^T @ ax gives the transposed sq), sums the 128 output partials per
core, scales by 1/V, averages the 8 per-core losses.
"""

import sys

sys.path.insert(0, "/opt/trn_rl_repo")

from contextlib import ExitStack

import ml_dtypes
import numpy as np

import concourse.bacc as bacc
import concourse.tile as tile
from concourse import mybir
from concourse.bass_utils import run_bass_kernel_spmd

BF16 = ml_dtypes.bfloat16

P = 128
V = 4096
H = V // 2  # quadrant width (2048)
KA = 24  # augmented contraction dim (3-way hi/mid/lo split)
NMM = 512  # matmul moving free dim (one fp32 PSUM bank)
NU = 16  # units (128-row blocks) per quadrant
T_SOFT = 0.004  # softmin temperature (valid min-sq range ~86*T = 0.344)
BIG = 1.0e30
N_C = 4  # Q21 units evacuated by DVE (ACT<->DVE rebalance)

_cache = {}


def _build_nc():
    F32 = mybir.dt.float32
    F16 = mybir.dt.float16
    BF = mybir.dt.bfloat16
    mn = mybir.AluOpType.min
    mx = mybir.AluOpType.max
    X = mybir.AxisListType.X
    AF = mybir.ActivationFunctionType

    nc = bacc.Bacc("TRN2", target_bir_lowering=False)
    ax_d = nc.declare_dram_parameter("ax", [KA, V], BF, isOutput=False)
    ay_d = nc.declare_dram_parameter("ay", [KA, V], BF, isOutput=False)
    idh_d = nc.declare_dram_parameter("identh", [P, P], F16, isOutput=False)
    svals_d = nc.declare_dram_parameter("svals", [P, 64], F32, isOutput=True)
    rawf_d = nc.declare_dram_parameter("rawf", [P, 64], F32, isOutput=True)

    with tile.TileContext(nc) as tc, ExitStack() as ctx:
        const = ctx.enter_context(tc.tile_pool(name="const", bufs=1))
        accs = ctx.enter_context(tc.tile_pool(name="accs", bufs=1))
        ecop = ctx.enter_context(tc.tile_pool(name="ecop", bufs=5))
        rcop = ctx.enter_context(tc.tile_pool(name="rcop", bufs=3))
        scratch = ctx.enter_context(tc.tile_pool(name="scratch", bufs=2))

        ax_sb = const.tile([KA, V], BF)
        ay_sb = const.tile([KA, V], BF)
        idh_sb = const.tile([P, P], F16)
        idb_sb = const.tile([P, P], BF)
        warmsrc = const.tile([1, 1], F32)
        warm = const.tile([1, 1], F32)
        nc.vector.memset(warmsrc[:], 1.0)
        # preload the Exp table set off the critical path
        nc.scalar.activation(warm[:], warmsrc[:], AF.Exp)
        nc.sync.dma_start(ax_sb[:, 0:P], ax_d[:, 0:P])
        nc.sync.dma_start(ay_sb[:, 0 : H // 2], ay_d[:, 0 : H // 2])
        nc.sync.dma_start(ax_sb[:, P:H], ax_d[:, P:H])
        nc.sync.dma_start(ay_sb[:, H // 2 : H], ay_d[:, H // 2 : H])
        nc.scalar.dma_start(ax_sb[:, H:V], ax_d[:, H:V])
        nc.scalar.dma_start(ay_sb[:, H:V], ay_d[:, H:V])
        # identities consumed only by the epilogue transposes
        nc.sync.dma_start(idh_sb[:], idh_d[:])
        nc.vector.tensor_copy(idb_sb[:], idh_sb[:])

        # col-chain accumulators (exp space, bf16) and raw min accumulators;
        # pairs share one tile so the epilogue transposes each in ONE DMA
        cacc_all = accs.tile([P, 2 * H], BF, name="cacc_all")
        racc_all = accs.tile([P, 2 * H], F16, name="racc_all")
        cacc11 = cacc_all[:, 0:H]  # max over Q11 units
        cacc22 = cacc_all[:, H : 2 * H]
        racc12 = racc_all[:, 0:H]  # min over Q12 units
        racc21 = racc_all[:, H : 2 * H]
        # svals2: paired per-group rowsum partials, [0:32] Q11, [32:64] Q22
        svals2 = accs.tile([P, 64], F32, name="svals2")
        rawc2 = accs.tile([P, max(2 * N_C, 1)], F32, name="rawc2")
        # svals: [0:16] Q11 rowsums, [16:32] Q22 rowsums,
        #        [32:48] Q11 colmax, [48:64] Q22 colmax
        svals = accs.tile([P, 64], F32, name="svals")
        # rawf: [0:32] x-row raw halves (transposed racc12|racc21),
        #       [32:48] Q21 fold mins, [48:64] Q12 fold mins
        rawf = accs.tile([P, 64], F32, name="rawf")
        work = accs.tile([P, 4 * 64], F32, name="work")

        ESCL = -1.0 / T_SOFT

        with tc.tile_pool(name="psum", bufs=4, space="PSUM") as psum:
            # one round = one unit from each quadrant; folds batch 4 rounds
            rawq = {"q12": [], "q21": []}
            gtile = {}
            for r in range(NU):
                units = [
                    ("q11", ax_sb[:, r * P : (r + 1) * P], ay_sb[:, 0:H], cacc11, True),
                    ("q21", ay_sb[:, r * P : (r + 1) * P], ax_sb[:, H:V], racc21, False),
                    ("q22", ax_sb[:, H + r * P : H + (r + 1) * P], ay_sb[:, H:V], cacc22, True),
                    ("q12", ay_sb[:, H + r * P : H + (r + 1) * P], ax_sb[:, 0:H], racc12, False),
                ]
                if r == NU - 1:
                    units = [units[1], units[3], units[0], units[2]]
                for qi, (qn, lhsT, rhs, acc, is_exp) in enumerate(units):
                    if is_exp:
                        ct = ecop.tile([P, H], BF, name="cte", tag="cte")
                    else:
                        is_c = qn == "q21" and r < N_C
                        if r % 4 == 0:
                            rawq[qn] = []
                            gtile[qn] = rcop.tile(
                                [P, 4 * H], F16, name="ctr", tag=f"ctr{qn}"
                            )
                        ct = gtile[qn]
                        ctu = ct[:, (r % 4) * H : (r % 4 + 1) * H]
                    # half-unit PSUM groups (2 banks) for finer PE/evac overlap
                    for g in range(2):
                        HG = H // 2
                        pst = psum.tile([P, HG], F32, name="ps", tag="ps")
                        for j in range(HG // NMM):
                            nc.tensor.matmul(
                                pst[:, j * NMM : (j + 1) * NMM],
                                lhsT,
                                rhs[:, g * HG + j * NMM : g * HG + (j + 1) * NMM],
                                start=True,
                                stop=True,
                                skip_group_check=True,
                            )
                        if is_exp:
                            # soft half: exp evac + free fused row-sum partial
                            slot = (0 if qn == "q11" else 32) + 2 * r + g
                            nc.scalar.activation(
                                ct[:, g * HG : (g + 1) * HG], pst[:], AF.Exp,
                                scale=ESCL,
                                accum_out=svals2[:, slot : slot + 1],
                            )
                        elif is_c:
                            # DVE evacuation with fused min-accum row fold
                            ci = r
                            nc.vector.tensor_scalar(
                                out=ctu[:, g * HG : (g + 1) * HG], in0=pst[:],
                                scalar1=BIG, scalar2=None, op0=mn, op1=mn,
                                accum_out=rawc2[:, 2 * ci + g : 2 * ci + g + 1],
                            )
                        else:
                            nc.scalar.copy(ctu[:, g * HG : (g + 1) * HG], pst[:])
                    if is_exp:
                        if r == 0:
                            nc.vector.tensor_copy(acc, ct[:])
                        else:
                            nc.vector.tensor_tensor(acc, ct[:], acc, mx)
                    else:
                        if not is_c and r < NU - 4:
                            rawq[qn].append((r, ct))
                        if r == 0:
                            nc.vector.tensor_copy(acc, ctu)
                        else:
                            nc.vector.tensor_tensor(acc, ctu, acc, mn)
                        if not is_c and r >= NU - 4:
                            # tail group: per-unit fold so only the last
                            # unit's tree lands after the final col TT
                            scu = scratch.tile(
                                [P, H // 2], F16, name="scu", tag="scu"
                            )
                            nc.vector.tensor_tensor(
                                scu[:], ctu[:, : H // 2], ctu[:, H // 2 :], mn
                            )
                            nc.vector.tensor_tensor(
                                scu[:, : H // 4], scu[:, : H // 4],
                                scu[:, H // 4 : H // 2], mn,
                            )
                            nc.vector.tensor_tensor(
                                scu[:, : H // 8], scu[:, : H // 8],
                                scu[:, H // 8 : H // 4], mn,
                            )
                            uslot = (32 if qn == "q21" else 48) + r
                            nc.vector.tensor_reduce(
                                rawf[:, uslot : uslot + 1], scu[:, : H // 8],
                                axis=X, op=mn,
                            )

                # batched fold trees for completed groups of 4 raw units
                if r % 4 == 3:
                    for qn in ("q21", "q12"):
                        grp = rawq[qn]
                        if not grp:
                            continue
                        r0, ct = grp[0][0], grp[0][1]
                        base = 32 if qn == "q21" else 48
                        c3 = ct[:].rearrange("p (m w) -> p m w", m=4)
                        scr = scratch.tile([P, 4 * (H // 2)], F16, name="scr", tag="scr")
                        s3 = scr[:].rearrange("p (m w) -> p m w", m=4)
                        nc.vector.tensor_tensor(
                            s3[:, :, :], c3[:, :, : H // 2], c3[:, :, H // 2 :], mn
                        )
                        nc.vector.tensor_tensor(
                            s3[:, :, : H // 4], s3[:, :, : H // 4],
                            s3[:, :, H // 4 : H // 2], mn,
                        )
                        nc.vector.tensor_tensor(
                            s3[:, :, : H // 8], s3[:, :, : H // 8],
                            s3[:, :, H // 8 : H // 4], mn,
                        )
                        nc.vector.tensor_reduce(
                            rawf[:, base + r0 : base + r0 + 4],
                            s3[:, :, : H // 8],
                            axis=X,
                            op=mn,
                        )

        # Epilogue: chunk-transpose the 4 accumulators via the DMA XBAR
        # (keeps the saturated PE out of the tail), partition-reduce, combine
        if True:
            tpr = accs.tile([P, 2 * H], F16, name="tpr")
            tpe = accs.tile([P, 2 * H], BF, name="tpe")
            # transpose-independent combines first: DVE overlaps the DMAs
            nc.vector.tensor_reduce(
                svals[:, 0:32], svals2[:].rearrange("p (a b) -> p a b", a=32),
                axis=X, op=mybir.AluOpType.add,
            )
            if N_C:
                nc.vector.tensor_reduce(
                    rawf[:, 32 : 32 + N_C],
                    rawc2[:, 0 : 2 * N_C].rearrange("p (a b) -> p a b", a=N_C),
                    axis=X, op=mn,
                )
            nc.sync.dma_start_transpose(
                tpr[:].rearrange("p (c j) -> p c j", c=2 * NU), racc_all[:]
            )
            nc.sync.dma_start_transpose(
                tpe[:].rearrange("p (c j) -> p c j", c=2 * NU), cacc_all[:]
            )
            # x-row raw halves: [0:16] v<2048 (racc12), [16:32] v>=2048
            # (racc21); 2x fold of the chunk interiors, then a 1x reduce
            t3r = tpr[:].rearrange("p (a b) -> p a b", a=2 * NU)
            nc.vector.tensor_tensor(
                t3r[:, :, 0:64], t3r[:, :, 0:64], t3r[:, :, 64:128], mn
            )
            nc.vector.tensor_reduce(
                rawf[:, 0:32], t3r[:, :, 0:64], axis=X, op=mn
            )
            # y-col soft halves: [32:48] w<2048 (cacc11), [48:64] w>=2048
            t3e = tpe[:].rearrange("p (a b) -> p a b", a=2 * NU)
            nc.vector.tensor_tensor(
                t3e[:, :, 0:64], t3e[:, :, 0:64], t3e[:, :, 64:128], mx
            )
            nc.vector.tensor_reduce(
                svals[:, 32:64], t3e[:, :, 0:64], axis=X, op=mx
            )

            nc.sync.dma_start(svals_d[:], svals[:])
            nc.scalar.dma_start(rawf_d[:], rawf[:])

    nc.finalize()
    return nc


def _split3(v):
    """3-way bf16 split: v ~= h + m + l with residual ~2^-27 |v|."""
    f32 = np.float32
    h = v.astype(BF16)
    m = (v - h.astype(f32)).astype(BF16)
    l = (v - h.astype(f32) - m.astype(f32)).astype(BF16)
    return h, m, l


def _augment(x, y):
    """x, y: (V, 3) fp32 -> AX, AY [24, V] bf16 3-way-split gram operands.

    sq = x2 + y2 + x.(-2y); products kept: hh, hm, mh, hl, lh, mm
    (magnitude >= ~2^-16); x2/y2 carried as 3 bf16 rows each. Symmetric:
    ay^T @ ax yields the transposed sq, so the same operands serve both
    gram orientations.
    """
    f32 = np.float32
    yy = (-2.0 * y).astype(f32)
    xh, xm, xl = _split3(x)
    yh, ym, yl = _split3(yy)
    x2 = np.einsum("vc,vc->v", x.astype(np.float64), x.astype(np.float64)).astype(f32)
    y2 = np.einsum("vc,vc->v", y.astype(np.float64), y.astype(np.float64)).astype(f32)
    x2h, x2m, x2l = _split3(x2)
    y2h, y2m, y2l = _split3(y2)
    one = np.ones(V, dtype=BF16)

    def cols(a):
        return [a[:, 0], a[:, 1], a[:, 2]]

    ax = np.stack(
        cols(xh) + cols(xh) + cols(xm) + cols(xh) + cols(xl) + cols(xm)
        + [x2h, x2m, x2l, one, one, one]
    )
    ay = np.stack(
        cols(yh) + cols(ym) + cols(yh) + cols(yl) + cols(yh) + cols(ym)
        + [one, one, one, y2h, y2m, y2l]
    )
    return ax, ay


def kernel(x, y):
    x = np.asarray(x, dtype=np.float32)
    y = np.asarray(y, dtype=np.float32)
    n = x.shape[0]
    assert x.shape == (n, V, 3) and y.shape == (n, V, 3) and n == 8

    if "nc" not in _cache:
        _cache["nc"] = _build_nc()
    nc = _cache["nc"]

    identh = np.eye(P, dtype=np.float16)
    in_maps = []
    for i in range(n):
        ax, ay = _augment(x[i], y[i])
        in_maps.append({"ax": ax, "ay": ay, "identh": identh})

    res = run_bass_kernel_spmd(
        nc, in_maps, list(range(n)), trace=_cache.get("trace", False)
    )
    _cache["last"] = res
    vals = []
    for i in range(n):
        s = np.asarray(res.results[i]["svals"], dtype=np.float64)
        raw = np.asarray(res.results[i]["rawf"], dtype=np.float64)
        soft = np.where(s > 1e-36, -T_SOFT * np.log(np.maximum(s, 1e-300)), np.inf)
        fin = np.maximum(np.minimum(soft, raw), 0.0)
        vals.append(np.sqrt(fin).sum() / V)
    return np.asarray(np.mean(vals), dtype=np.float32)



# revision 2
# speedup vs baseline: 1.0019x; 1.0019x over previous
"""Chamfer loss Trainium2 kernel v2 (data-parallel over batch, 8 NeuronCores).

Per-core architecture ("three-region pinline"), replacing the v1 quadrant
scheme. The 4096x4096 sq-distance matrix is split into three w-regions:

  X  (w <  2048)  x-major SOFT: ACT exp evac (T=0.0015) + free accum_out
                  row-sums; column covers via PE ones-matmul col-SUMS
                  accumulated in one resident PSUM bank (4 x 512 chunks at
                  col-tile positions 0/32/64/96, M=1).
  Y  (w in [2048,2560)) y-major SOFT strip: ACT exp evac + free accum_out
                  col-sums; row covers via DVE bf16 exp-MAX chains over the
                  4 w-blocks (exact selection in exp space) + transposed
                  folds.
  R  (w >= 2560)  y-major RAW: DVE tensor_scalar evac (fp16, min BIG) with
                  fused accum_out col-mins (exact), cross-w-block fp16 min
                  chains for row covers + transposed folds.

Row v = min(softmin_X, strip_expmax_Y, raw_R); col w from its region's
cover. Softmin underflow falls back to the other covers (rows) or an
85T cap (cols). The PE runs 2/3-way row-tiled K=24 matmuls
(tile_position) plus 4-way col-tiled M=1 colsum matmuls; engines pace
the pipeline (~92-100us each by the cost model), PE is ~70us cold.

PSUM budget (8 banks, exact): C colsum [128,512] resident (1 bank) +
S soft tile [128,2048] (4) + B raw tile [128,1536] (3); S and B are
single-buffered with chase-split drains: each drain half's evacuation
overlaps the matmuls refilling the other half.

Host: packs error-compensated 3-way bf16 split gram operands replicated
at partition groups 0/32/64/96 (row-tiling reads lhsT/rhs from the
matching 32-partition group), reassembles rows/cols, averages the 8
per-core losses.
"""

import sys

sys.path.insert(0, "/opt/trn_rl_repo")

from contextlib import ExitStack

import ml_dtypes
import numpy as np

import concourse.bacc as bacc
import concourse.tile as tile
from concourse import mybir
from concourse.bass_utils import run_bass_kernel_spmd

BF16 = ml_dtypes.bfloat16

P = 128
V = 4096
KA = 24
WX = 2048  # x-major soft region width
WY = 512  # y-major soft strip width (4 w-blocks)
T_SOFT = 0.0015
BIG = 1.0e30

_cache = {}

# raw region v-panels
RP_OFF = [0, 1536, 3072]
RP_W = [1536, 1536, 1024]


def _build_nc():
    F32 = mybir.dt.float32
    F16 = mybir.dt.float16
    BF = mybir.dt.bfloat16
    mn = mybir.AluOpType.min
    mx = mybir.AluOpType.max
    X = mybir.AxisListType.X
    AF = mybir.ActivationFunctionType

    nc = bacc.Bacc("TRN2", target_bir_lowering=False)
    ax_d = nc.declare_dram_parameter("ax", [P, V], BF, isOutput=False)
    ay_d = nc.declare_dram_parameter("ay", [P, V], BF, isOutput=False)
    svals_d = nc.declare_dram_parameter("svals", [P, 80], F32, isOutput=True)
    csv_d = nc.declare_dram_parameter("csv", [P, 512], F32, isOutput=True)
    rawc_d = nc.declare_dram_parameter("rawc", [P, 80], F32, isOutput=True)
    rawf_d = nc.declare_dram_parameter("rawf", [P, 80], F32, isOutput=True)

    ESCL = -1.0 / T_SOFT

    with tile.TileContext(nc) as tc, ExitStack() as ctx:
        const = ctx.enter_context(tc.tile_pool(name="const", bufs=1))
        accs = ctx.enter_context(tc.tile_pool(name="accs", bufs=1))
        ecop = ctx.enter_context(tc.tile_pool(name="ecop", bufs=3))
        ycop = ctx.enter_context(tc.tile_pool(name="ycop", bufs=2))
        rcop = ctx.enter_context(tc.tile_pool(name="rcop", bufs=3))
        tpp = ctx.enter_context(tc.tile_pool(name="tpp", bufs=2))

        ax_sb = const.tile([P, V], BF)
        ay_sb = const.tile([P, V], BF)
        ones1 = const.tile([P, 1], BF)
        warmsrc = const.tile([1, 1], F32)
        warm = const.tile([1, 1], F32)
        nc.vector.memset(ones1[:], 1.0)
        nc.vector.memset(warmsrc[:], 1.0)
        # input DMAs first (sync queue -- the scalar queue is blocked ~4us
        # by the exp table load), ordered by first consumption: soft-X b=0
        # needs ay0-3 + ax0; the raw stream needs ay5 + ax0-2 right away.
        _order = [("y", 0), ("x", 0), ("y", 5), ("y", 1), ("x", 1), ("y", 2),
                  ("x", 2), ("y", 6), ("y", 3), ("x", 3), ("y", 4), ("y", 7),
                  ("x", 4), ("x", 5), ("x", 6), ("x", 7)]
        for which, k in _order:
            o = 512 * k
            src, dst = (ay_d, ay_sb) if which == "y" else (ax_d, ax_sb)
            nc.sync.dma_start(dst[:, o : o + 512], src[:, o : o + 512])
        nc.scalar.activation(warm[:], warmsrc[:], AF.Exp)

        # accumulators / outputs
        # svals: [0:64] soft-X rowsum halves (2 per v-block);
        #        [64:72] strip colsum (h=0, slot 64+2wb+q);
        #        [72:80] strip colsum (h=1, slot 72+2wb+q)
        svals = accs.tile([P, 80], F32, name="svals")
        csv = accs.tile([P, 512], F32, name="csv")
        # rawc: raw col partials, slot 12q+wb
        rawc = accs.tile([P, 80], F32, name="rawc")
        # rawf: [0:12] raw rows q0, [12:24] q1, [24:32] q2;
        #       [32:48] strip rows q0 (expmax), [48:64] strip rows q1
        rawf = accs.tile([P, 80], F32, name="rawf")
        yracc = accs.tile([P, 2 * 2048], BF, name="yracc")
        rracc = accs.tile([P, V], F16, name="rracc")

        with tc.tile_pool(name="psumC", bufs=1, space="PSUM") as psumC, \
             tc.tile_pool(name="psumS", bufs=2, space="PSUM") as psumS, \
             tc.tile_pool(name="psumB", bufs=1, space="PSUM") as psumB:
            C = psumC.tile([P, 512], F32, name="C")
            B = psumB.tile([P, 1536], F32, name="B")

            # S-stream jobs: soft-X v-blocks with strip jobs mixed in evenly
            xjobs = [("x", b) for b in range(32)]
            yjobs = [("y", (wb, q)) for q in range(2) for wb in range(4)]
            sstream = []
            yi = iter(yjobs)
            for b in range(32):
                sstream.append(xjobs[b])
                if b % 3 == 2:
                    nx = next(yi, None)
                    if nx is not None:
                        sstream.append(nx)
            sstream.extend(yi)
            rjobs = [("r", (wb, q)) for q in range(3) for wb in range(12)]

            # interleave S-stream (40 jobs) with B-stream (36 jobs) ~1:1
            sjobs = []
            si = iter(sstream)
            for i, rj in enumerate(rjobs):
                sjobs.append(rj)
                for _ in range(2 if i % 9 == 0 else 1):
                    nx = next(si, None)
                    if nx is not None:
                        sjobs.append(nx)
            for nx in si:
                sjobs.append(nx)

            def soft_job(ct, lhs_ap, rhs_base, rhs_off, slots):
                """Two [128,1024] S-pool tiles; all 4 MMs 4-way row-tiled
                (groups 0-3) issued before the two exp evacs."""
                tiles = [psumS.tile([P, 1024], F32, name="S", tag="S")
                         for _ in range(2)]
                for h in range(2):
                    for j in range(2):
                        g = 2 * h + j
                        o = h * 1024 + j * 512
                        nc.tensor.matmul(
                            tiles[h][:, j * 512 : (j + 1) * 512],
                            lhs_ap(g),
                            rhs_base(g)[:, rhs_off + o : rhs_off + o + 512],
                            start=True, stop=True, skip_group_check=True,
                            tile_position=(32 * g, 0),
                        )
                for h in range(2):
                    nc.scalar.activation(
                        ct[:, h * 1024 : (h + 1) * 1024], tiles[h][:],
                        AF.Exp, scale=ESCL,
                        accum_out=svals[:, slots[h] : slots[h] + 1],
                    )

            pending_cs = []  # delayed colsum emissions: (ct, b)

            def emit_pending_colsums(keep):
                while len(pending_cs) > keep:
                    ct, b, first, last = pending_cs.pop(0)
                    for c in range(4):
                        nc.tensor.matmul(
                            C[32 * c : 32 * c + 1, 0:512],
                            ones1[:],
                            ct[:, c * 512 : (c + 1) * 512],
                            start=first, stop=last,
                            skip_group_check=True,
                            tile_position=(0, 32 * c),
                        )
                    if last:
                        nc.scalar.copy(csv[:], C[:])
                        nc.scalar.dma_start(csv_d[:], csv[:])

            nx = 0
            for job, arg in sjobs:
                if job == "x":
                    b = arg
                    ct = ecop.tile([P, 2048], BF, name="ctx", tag="ctx")
                    soft_job(
                        ct,
                        lambda g: ax_sb[32 * g : 32 * g + KA, b * P : (b + 1) * P],
                        lambda g: ay_sb[32 * g : 32 * g + KA, :],
                        0, (2 * b, 2 * b + 1),
                    )
                    pending_cs.append((ct, b, nx == 0, nx == 31))
                    nx += 1
                    emit_pending_colsums(keep=1 if nx < 32 else 0)
                elif job == "y":
                    wb, q = arg
                    if wb == 0:
                        yct = yracc[:, q * 2048 : (q + 1) * 2048]
                    else:
                        ytile = ycop.tile([P, 2048], BF, name="cty", tag="cty")
                        yct = ytile[:]
                    soft_job(
                        yct,
                        lambda g: ay_sb[32 * g : 32 * g + KA,
                                        WX + wb * P : WX + (wb + 1) * P],
                        lambda g: ax_sb[32 * g : 32 * g + KA, :],
                        q * 2048,
                        (64 + 2 * wb + q, 72 + 2 * wb + q),
                    )
                    if wb != 0:
                        nc.vector.tensor_tensor(
                            yracc[:, q * 2048 : (q + 1) * 2048], yct,
                            yracc[:, q * 2048 : (q + 1) * 2048], mx,
                        )
                    if wb == 3:
                        # strip panel complete: transpose + expmax fold
                        tpy = tpp.tile([P, 2048], BF, name="tpy", tag="tpy")
                        nc.sync.dma_start_transpose(
                            tpy[:].rearrange("p (c j) -> p c j", c=16),
                            yracc[:, q * 2048 : (q + 1) * 2048],
                        )
                        t3 = tpy[:].rearrange("p (c j) -> p c j", c=16)
                        nc.vector.tensor_tensor(
                            t3[:, :, 0:64], t3[:, :, 0:64], t3[:, :, 64:128], mx
                        )
                        nc.vector.tensor_tensor(
                            t3[:, :, 0:32], t3[:, :, 0:32], t3[:, :, 32:64], mx
                        )
                        nc.vector.tensor_reduce(
                            rawf[:, 32 + 16 * q : 48 + 16 * q], t3[:, :, 0:32],
                            axis=X, op=mx,
                        )
                        nc.sync.dma_start(
                            rawf_d[:, 32 + 16 * q : 48 + 16 * q],
                            rawf[:, 32 + 16 * q : 48 + 16 * q],
                        )
                else:  # raw
                    wb, q = arg
                    off, w = RP_OFF[q], RP_W[q]
                    for j in range(w // 512):
                        g = (j + 2 * (wb % 2)) % 4
                        nc.tensor.matmul(
                            B[:, j * 512 : (j + 1) * 512],
                            ay_sb[32 * g : 32 * g + KA,
                                  WX + WY + wb * P : WX + WY + (wb + 1) * P],
                            ax_sb[32 * g : 32 * g + KA,
                                  off + j * 512 : off + (j + 1) * 512],
                            start=True, stop=True, skip_group_check=True,
                            tile_position=(32 * g, 0),
                        )
                    if wb == 0:
                        rt = rracc[:, off : off + w]
                    else:
                        rtile = rcop.tile([P, 1536], F16, name="ctr", tag="ctr")
                        rt = rtile[:, 0:w]
                    s0 = 12 * q + wb
                    nc.vector.tensor_scalar(
                        out=rt[:], in0=B[:, 0:w],
                        scalar1=BIG, scalar2=None, op0=mn, op1=mn,
                        accum_out=rawc[:, s0 : s0 + 1],
                    )
                    if wb != 0:
                        nc.vector.tensor_tensor(
                            rracc[:, off : off + w], rt,
                            rracc[:, off : off + w], mn,
                        )
                    if wb == 11:
                        nch = w // P
                        tpr = tpp.tile([P, 1536], F16, name="tpr", tag="tpr")
                        nc.sync.dma_start_transpose(
                            tpr[:, 0:w].rearrange("p (c j) -> p c j", c=nch),
                            rracc[:, off : off + w],
                        )
                        t3 = tpr[:, 0:w].rearrange("p (c j) -> p c j", c=nch)
                        nc.vector.tensor_tensor(
                            t3[:, :, 0:64], t3[:, :, 0:64], t3[:, :, 64:128], mn
                        )
                        nc.vector.tensor_tensor(
                            t3[:, :, 0:32], t3[:, :, 0:32], t3[:, :, 32:64], mn
                        )
                        fs = [0, 12, 24][q]
                        nc.vector.tensor_reduce(
                            rawf[:, fs : fs + nch], t3[:, :, 0:32], axis=X, op=mn
                        )
                        nc.sync.dma_start(
                            rawf_d[:, fs : fs + nch], rawf[:, fs : fs + nch]
                        )

        nc.sync.dma_start(svals_d[:], svals[:])
        nc.sync.dma_start(rawc_d[:], rawc[:])

    nc.finalize()
    return nc


def _split3(v):
    f32 = np.float32
    h = v.astype(BF16)
    m = (v - h.astype(f32)).astype(BF16)
    l = (v - h.astype(f32) - m.astype(f32)).astype(BF16)
    return h, m, l


def _augment(x, y):
    """(V,3) fp32 -> AX, AY [24, V] bf16 split gram operands (symmetric)."""
    f32 = np.float32
    yy = (-2.0 * y).astype(f32)
    xh, xm, xl = _split3(x)
    yh, ym, yl = _split3(yy)
    x2 = np.einsum("vc,vc->v", x.astype(np.float64), x.astype(np.float64)).astype(f32)
    y2 = np.einsum("vc,vc->v", y.astype(np.float64), y.astype(np.float64)).astype(f32)
    x2h, x2m, x2l = _split3(x2)
    y2h, y2m, y2l = _split3(y2)
    one = np.ones(V, dtype=BF16)

    def cols(a):
        return [a[:, 0], a[:, 1], a[:, 2]]

    ax = np.stack(
        cols(xh) + cols(xh) + cols(xm) + cols(xh) + cols(xl) + cols(xm)
        + [x2h, x2m, x2l, one, one, one]
    )
    ay = np.stack(
        cols(yh) + cols(ym) + cols(yh) + cols(yl) + cols(yh) + cols(ym)
        + [one, one, one, y2h, y2m, y2l]
    )
    return ax, ay


def _replicate(a):
    out = np.zeros((P, V), dtype=BF16)
    for g in range(4):
        out[32 * g : 32 * g + KA] = a
    return out


def kernel(x, y):
    x = np.asarray(x, dtype=np.float32)
    y = np.asarray(y, dtype=np.float32)
    n = x.shape[0]
    assert x.shape == (n, V, 3) and y.shape == (n, V, 3) and n == 8

    if "nc" not in _cache:
        _cache["nc"] = _build_nc()
    nc = _cache["nc"]

    in_maps = []
    for i in range(n):
        ax, ay = _augment(x[i], y[i])
        in_maps.append({"ax": _replicate(ax), "ay": _replicate(ay)})

    res = run_bass_kernel_spmd(
        nc, in_maps, list(range(n)), trace=_cache.get("trace", False)
    )
    _cache["last"] = res

    T = T_SOFT
    CAP = 85.0 * T
    BIGF = 1.0e30
    TINY = 1e-35
    vals = []
    for i in range(n):
        r = res.results[i]
        svals = np.asarray(r["svals"], dtype=np.float64)
        csv = np.asarray(r["csv"], dtype=np.float64)
        rawc = np.asarray(r["rawc"], dtype=np.float64)
        rawf = np.asarray(r["rawf"], dtype=np.float64)

        def softsq(s, fb):
            return np.where(s > TINY, -T * np.log(np.maximum(s, 1e-300)), fb)

        # ---- rows (layout [p, b] with v = 128b + p) ----
        rs = svals[:, 0:64:2] + svals[:, 1:64:2]  # [128, 32]
        rowsoft = softsq(rs, BIGF)
        rowstrip = np.concatenate(
            [softsq(rawf[:, 32:48], BIGF), softsq(rawf[:, 48:64], BIGF)], axis=1
        )
        rowraw = np.concatenate(
            [rawf[:, 0:12], rawf[:, 12:24], rawf[:, 24:32]], axis=1
        )
        row_sq = np.maximum(
            np.minimum(np.minimum(rowsoft, rowstrip), rowraw), 0.0
        )
        row_d = np.sqrt(row_sq)

        # ---- cols ----
        colsum = np.concatenate([csv[32 * c, 0:512] for c in range(4)])
        colsoft = softsq(colsum, CAP)  # w < 2048
        colstrip = np.zeros((P, 4))
        for wb in range(4):
            colstrip[:, wb] = (
                svals[:, 64 + 2 * wb] + svals[:, 64 + 2 * wb + 1]
                + svals[:, 72 + 2 * wb] + svals[:, 72 + 2 * wb + 1]
            )
        colstrip_sq = softsq(colstrip, CAP)  # [p, wb], w = 2048 + 128wb + p
        colraw = np.zeros((P, 12))
        for wb in range(12):
            colraw[:, wb] = np.minimum.reduce(
                [rawc[:, 12 * q + wb] for q in range(3)]
            )
        col_d = np.concatenate([
            np.sqrt(np.maximum(colsoft, 0.0)),
            np.sqrt(np.maximum(colstrip_sq, 0.0)).T.reshape(-1),
            np.sqrt(np.maximum(colraw, 0.0)).T.reshape(-1),
        ])
        vals.append(row_d.mean() + col_d.mean())
    return np.asarray(np.mean(vals), dtype=np.float32)


# revision 3
# speedup vs baseline: 1.0038x; 1.0019x over previous
"""Chamfer loss Trainium2 kernel v2 (data-parallel over batch, 8 NeuronCores).

Per-core architecture ("three-region pinline"), replacing the v1 quadrant
scheme. The 4096x4096 sq-distance matrix is split into three w-regions:

  X  (w <  2048)  x-major SOFT: ACT exp evac (T=0.0015) + free accum_out
                  row-sums; column covers via PE ones-matmul col-SUMS
                  accumulated in one resident PSUM bank (4 x 512 chunks at
                  col-tile positions 0/32/64/96, M=1).
  Y  (w in [2048,2560)) y-major SOFT strip: ACT exp evac + free accum_out
                  col-sums; row covers via DVE bf16 exp-MAX chains over the
                  4 w-blocks (exact selection in exp space) + transposed
                  folds.
  R  (w >= 2560)  y-major RAW: DVE tensor_scalar evac (fp16, min BIG) with
                  fused accum_out col-mins (exact), cross-w-block fp16 min
                  chains for row covers + transposed folds.

Row v = min(softmin_X, strip_expmax_Y, raw_R); col w from its region's
cover. Softmin underflow falls back to the other covers (rows) or an
85T cap (cols). The PE runs 2/3-way row-tiled K=24 matmuls
(tile_position) plus 4-way col-tiled M=1 colsum matmuls; engines pace
the pipeline (~92-100us each by the cost model), PE is ~70us cold.

PSUM budget (8 banks, exact): C colsum [128,512] resident (1 bank) +
S soft tile [128,2048] (4) + B raw tile [128,1536] (3); S and B are
single-buffered with chase-split drains: each drain half's evacuation
overlaps the matmuls refilling the other half.

Host: packs error-compensated 3-way bf16 split gram operands replicated
at partition groups 0/32/64/96 (row-tiling reads lhsT/rhs from the
matching 32-partition group), reassembles rows/cols, averages the 8
per-core losses.
"""

import sys

sys.path.insert(0, "/opt/trn_rl_repo")

from contextlib import ExitStack

import ml_dtypes
import numpy as np

import concourse.bacc as bacc
import concourse.tile as tile
from concourse import mybir
from concourse.bass_utils import run_bass_kernel_spmd

BF16 = ml_dtypes.bfloat16

P = 128
V = 4096
KA = 24
WX = 2048  # x-major soft region width
WY = 512  # y-major soft strip width (4 w-blocks)
T_SOFT = 0.0015
BIG = 1.0e30

_cache = {}

# raw region v-panels
RP_OFF = [0, 1536, 3072]
RP_W = [1536, 1536, 1024]


def _build_nc():
    F32 = mybir.dt.float32
    F16 = mybir.dt.float16
    BF = mybir.dt.bfloat16
    mn = mybir.AluOpType.min
    mx = mybir.AluOpType.max
    X = mybir.AxisListType.X
    AF = mybir.ActivationFunctionType

    nc = bacc.Bacc("TRN2", target_bir_lowering=False)
    ax_d = nc.declare_dram_parameter("ax", [P, V], BF, isOutput=False)
    ay_d = nc.declare_dram_parameter("ay", [P, V], BF, isOutput=False)
    svals_d = nc.declare_dram_parameter("svals", [P, 80], F32, isOutput=True)
    csv_d = nc.declare_dram_parameter("csv", [P, 512], F32, isOutput=True)
    rawc_d = nc.declare_dram_parameter("rawc", [P, 80], F32, isOutput=True)
    rawf_d = nc.declare_dram_parameter("rawf", [P, 80], F32, isOutput=True)

    ESCL = -1.0 / T_SOFT

    with tile.TileContext(nc) as tc, ExitStack() as ctx:
        const = ctx.enter_context(tc.tile_pool(name="const", bufs=1))
        accs = ctx.enter_context(tc.tile_pool(name="accs", bufs=1))
        ecop = ctx.enter_context(tc.tile_pool(name="ecop", bufs=3))
        ycop = ctx.enter_context(tc.tile_pool(name="ycop", bufs=2))
        rcop = ctx.enter_context(tc.tile_pool(name="rcop", bufs=3))
        tpp = ctx.enter_context(tc.tile_pool(name="tpp", bufs=2))

        ax_sb = const.tile([P, V], BF)
        ay_sb = const.tile([P, V], BF)
        ones1 = const.tile([P, 1], BF)
        warmsrc = const.tile([1, 1], F32)
        warm = const.tile([1, 1], F32)
        nc.vector.memset(ones1[:], 1.0)
        nc.vector.memset(warmsrc[:], 1.0)
        # input DMAs first (sync queue -- the scalar queue is blocked ~4us
        # by the exp table load), ordered by first consumption: soft-X b=0
        # needs ay0-3 + ax0; the raw stream needs ay5 + ax0-2 right away.
        # first chunks split small so the first matmuls start ASAP
        nc.sync.dma_start(ay_sb[:, 0:256], ay_d[:, 0:256])
        nc.sync.dma_start(ax_sb[:, 0:256], ax_d[:, 0:256])
        nc.sync.dma_start(ay_sb[:, 2560:2816], ay_d[:, 2560:2816])
        nc.sync.dma_start(ay_sb[:, 256:512], ay_d[:, 256:512])
        nc.sync.dma_start(ax_sb[:, 256:512], ax_d[:, 256:512])
        nc.sync.dma_start(ay_sb[:, 2816:3072], ay_d[:, 2816:3072])
        _order = [("y", 1), ("x", 1), ("y", 2), ("x", 2), ("y", 6), ("y", 3),
                  ("x", 3), ("y", 4), ("y", 7), ("x", 4), ("x", 5), ("x", 6),
                  ("x", 7)]
        for which, k in _order:
            o = 512 * k
            src, dst = (ay_d, ay_sb) if which == "y" else (ax_d, ax_sb)
            nc.sync.dma_start(dst[:, o : o + 512], src[:, o : o + 512])
        nc.scalar.activation(warm[:], warmsrc[:], AF.Exp)

        # accumulators / outputs
        # svals: [0:64] soft-X rowsum halves (2 per v-block);
        #        [64:72] strip colsum (h=0, slot 64+2wb+q);
        #        [72:80] strip colsum (h=1, slot 72+2wb+q)
        svals = accs.tile([P, 80], F32, name="svals")
        csv = accs.tile([P, 512], F32, name="csv")
        # rawc: raw col partials, slot 12q+wb
        rawc = accs.tile([P, 80], F32, name="rawc")
        # rawf: [0:12] raw rows q0, [12:24] q1, [24:32] q2;
        #       [32:48] strip rows q0 (expmax), [48:64] strip rows q1
        rawf = accs.tile([P, 80], F32, name="rawf")
        yracc = accs.tile([P, 2 * 2048], BF, name="yracc")
        rracc = accs.tile([P, V], F16, name="rracc")

        with tc.tile_pool(name="psumC", bufs=1, space="PSUM") as psumC, \
             tc.tile_pool(name="psumS", bufs=2, space="PSUM") as psumS, \
             tc.tile_pool(name="psumB", bufs=1, space="PSUM") as psumB:
            C = psumC.tile([P, 512], F32, name="C")
            B = psumB.tile([P, 1536], F32, name="B")

            # S-stream jobs: soft-X v-blocks with strip jobs mixed in evenly
            xjobs = [("x", b) for b in range(32)]
            yjobs = [("y", (wb, q)) for q in range(2) for wb in range(4)]
            sstream = []
            yi = iter(yjobs)
            for b in range(32):
                sstream.append(xjobs[b])
                if b % 3 == 2:
                    nx = next(yi, None)
                    if nx is not None:
                        sstream.append(nx)
            sstream.extend(yi)
            rjobs = [("r", (wb, q)) for q in range(3) for wb in range(12)]

            # interleave S-stream (40 jobs) with B-stream (36 jobs) ~1:1;
            # lead with a raw job so the DVE stream starts immediately
            sjobs = [rjobs[0]]
            si = iter(sstream)
            for i, rj in enumerate(rjobs[1:]):
                sjobs.append(rj)
                for _ in range(2 if i % 8 == 0 else 1):
                    nx = next(si, None)
                    if nx is not None:
                        sjobs.append(nx)
            for nx in si:
                sjobs.append(nx)

            def soft_job(ct, lhs_ap, rhs_base, rhs_off, slots):
                """Two [128,1024] S-pool tiles; all 4 MMs 4-way row-tiled
                (groups 0-3) issued before the two exp evacs."""
                tiles = [psumS.tile([P, 1024], F32, name="S", tag="S")
                         for _ in range(2)]
                for h in range(2):
                    for j in range(2):
                        g = 2 * h + j
                        o = h * 1024 + j * 512
                        nc.tensor.matmul(
                            tiles[h][:, j * 512 : (j + 1) * 512],
                            lhs_ap(g),
                            rhs_base(g)[:, rhs_off + o : rhs_off + o + 512],
                            start=True, stop=True, skip_group_check=True,
                            tile_position=(32 * g, 0),
                        )
                for h in range(2):
                    nc.scalar.activation(
                        ct[:, h * 1024 : (h + 1) * 1024], tiles[h][:],
                        AF.Exp, scale=ESCL,
                        accum_out=svals[:, slots[h] : slots[h] + 1],
                    )

            pending_cs = []  # delayed colsum emissions: (ct, b)

            def emit_pending_colsums(keep):
                while len(pending_cs) > keep:
                    ct, b, first, last = pending_cs.pop(0)
                    for c in range(4):
                        nc.tensor.matmul(
                            C[32 * c : 32 * c + 1, 0:512],
                            ones1[:],
                            ct[:, c * 512 : (c + 1) * 512],
                            start=first, stop=last,
                            skip_group_check=True,
                            tile_position=(0, 32 * c),
                        )
                    if last:
                        nc.scalar.copy(csv[:], C[:])
                        nc.scalar.dma_start(csv_d[:], csv[:])

            nx = 0
            for job, arg in sjobs:
                if job == "x":
                    b = arg
                    ct = ecop.tile([P, 2048], BF, name="ctx", tag="ctx")
                    soft_job(
                        ct,
                        lambda g: ax_sb[32 * g : 32 * g + KA, b * P : (b + 1) * P],
                        lambda g: ay_sb[32 * g : 32 * g + KA, :],
                        0, (2 * b, 2 * b + 1),
                    )
                    pending_cs.append((ct, b, nx == 0, nx == 31))
                    nx += 1
                    emit_pending_colsums(keep=1 if nx < 32 else 0)
                elif job == "y":
                    wb, q = arg
                    if wb == 0:
                        yct = yracc[:, q * 2048 : (q + 1) * 2048]
                    else:
                        ytile = ycop.tile([P, 2048], BF, name="cty", tag="cty")
                        yct = ytile[:]
                    soft_job(
                        yct,
                        lambda g: ay_sb[32 * g : 32 * g + KA,
                                        WX + wb * P : WX + (wb + 1) * P],
                        lambda g: ax_sb[32 * g : 32 * g + KA, :],
                        q * 2048,
                        (64 + 2 * wb + q, 72 + 2 * wb + q),
                    )
                    if wb != 0:
                        nc.vector.tensor_tensor(
                            yracc[:, q * 2048 : (q + 1) * 2048], yct,
                            yracc[:, q * 2048 : (q + 1) * 2048], mx,
                        )
                    if wb == 3:
                        # strip panel complete: transpose + expmax fold
                        tpy = tpp.tile([P, 2048], BF, name="tpy", tag="tpy")
                        nc.sync.dma_start_transpose(
                            tpy[:].rearrange("p (c j) -> p c j", c=16),
                            yracc[:, q * 2048 : (q + 1) * 2048],
                        )
                        t3 = tpy[:].rearrange("p (c j) -> p c j", c=16)
                        nc.vector.tensor_tensor(
                            t3[:, :, 0:64], t3[:, :, 0:64], t3[:, :, 64:128], mx
                        )
                        nc.vector.tensor_tensor(
                            t3[:, :, 0:32], t3[:, :, 0:32], t3[:, :, 32:64], mx
                        )
                        nc.vector.tensor_reduce(
                            rawf[:, 32 + 16 * q : 48 + 16 * q], t3[:, :, 0:32],
                            axis=X, op=mx,
                        )
                        nc.sync.dma_start(
                            rawf_d[:, 32 + 16 * q : 48 + 16 * q],
                            rawf[:, 32 + 16 * q : 48 + 16 * q],
                        )
                else:  # raw
                    wb, q = arg
                    off, w = RP_OFF[q], RP_W[q]
                    for j in range(w // 512):
                        g = (j + 2 * (wb % 2)) % 4
                        nc.tensor.matmul(
                            B[:, j * 512 : (j + 1) * 512],
                            ay_sb[32 * g : 32 * g + KA,
                                  WX + WY + wb * P : WX + WY + (wb + 1) * P],
                            ax_sb[32 * g : 32 * g + KA,
                                  off + j * 512 : off + (j + 1) * 512],
                            start=True, stop=True, skip_group_check=True,
                            tile_position=(32 * g, 0),
                        )
                    if wb == 0:
                        rt = rracc[:, off : off + w]
                    else:
                        rtile = rcop.tile([P, 1536], F16, name="ctr", tag="ctr")
                        rt = rtile[:, 0:w]
                    s0 = 12 * q + wb
                    nc.vector.tensor_scalar(
                        out=rt[:], in0=B[:, 0:w],
                        scalar1=BIG, scalar2=None, op0=mn, op1=mn,
                        accum_out=rawc[:, s0 : s0 + 1],
                    )
                    if wb != 0:
                        nc.vector.tensor_tensor(
                            rracc[:, off : off + w], rt,
                            rracc[:, off : off + w], mn,
                        )
                    if wb == 11:
                        nch = w // P
                        tpr = tpp.tile([P, 1536], F16, name="tpr", tag="tpr")
                        nc.sync.dma_start_transpose(
                            tpr[:, 0:w].rearrange("p (c j) -> p c j", c=nch),
                            rracc[:, off : off + w],
                        )
                        t3 = tpr[:, 0:w].rearrange("p (c j) -> p c j", c=nch)
                        nc.vector.tensor_tensor(
                            t3[:, :, 0:64], t3[:, :, 0:64], t3[:, :, 64:128], mn
                        )
                        nc.vector.tensor_tensor(
                            t3[:, :, 0:32], t3[:, :, 0:32], t3[:, :, 32:64], mn
                        )
                        fs = [0, 12, 24][q]
                        nc.vector.tensor_reduce(
                            rawf[:, fs : fs + nch], t3[:, :, 0:32], axis=X, op=mn
                        )
                        nc.sync.dma_start(
                            rawf_d[:, fs : fs + nch], rawf[:, fs : fs + nch]
                        )

        nc.scalar.dma_start(svals_d[:], svals[:])
        nc.scalar.dma_start(rawc_d[:], rawc[:])

    nc.finalize()
    return nc


def _split3(v):
    f32 = np.float32
    h = v.astype(BF16)
    m = (v - h.astype(f32)).astype(BF16)
    l = (v - h.astype(f32) - m.astype(f32)).astype(BF16)
    return h, m, l


def _augment(x, y):
    """(V,3) fp32 -> AX, AY [24, V] bf16 split gram operands (symmetric)."""
    f32 = np.float32
    yy = (-2.0 * y).astype(f32)
    xh, xm, xl = _split3(x)
    yh, ym, yl = _split3(yy)
    x2 = np.einsum("vc,vc->v", x.astype(np.float64), x.astype(np.float64)).astype(f32)
    y2 = np.einsum("vc,vc->v", y.astype(np.float64), y.astype(np.float64)).astype(f32)
    x2h, x2m, x2l = _split3(x2)
    y2h, y2m, y2l = _split3(y2)
    one = np.ones(V, dtype=BF16)

    def cols(a):
        return [a[:, 0], a[:, 1], a[:, 2]]

    ax = np.stack(
        cols(xh) + cols(xh) + cols(xm) + cols(xh) + cols(xl) + cols(xm)
        + [x2h, x2m, x2l, one, one, one]
    )
    ay = np.stack(
        cols(yh) + cols(ym) + cols(yh) + cols(yl) + cols(yh) + cols(ym)
        + [one, one, one, y2h, y2m, y2l]
    )
    return ax, ay


def _replicate(a):
    out = np.zeros((P, V), dtype=BF16)
    for g in range(4):
        out[32 * g : 32 * g + KA] = a
    return out


def kernel(x, y):
    x = np.asarray(x, dtype=np.float32)
    y = np.asarray(y, dtype=np.float32)
    n = x.shape[0]
    assert x.shape == (n, V, 3) and y.shape == (n, V, 3) and n == 8

    if "nc" not in _cache:
        _cache["nc"] = _build_nc()
    nc = _cache["nc"]

    in_maps = []
    for i in range(n):
        ax, ay = _augment(x[i], y[i])
        in_maps.append({"ax": _replicate(ax), "ay": _replicate(ay)})

    res = run_bass_kernel_spmd(
        nc, in_maps, list(range(n)), trace=_cache.get("trace", False)
    )
    _cache["last"] = res

    T = T_SOFT
    CAP = 85.0 * T
    BIGF = 1.0e30
    TINY = 1e-35
    vals = []
    for i in range(n):
        r = res.results[i]
        svals = np.asarray(r["svals"], dtype=np.float64)
        csv = np.asarray(r["csv"], dtype=np.float64)
        rawc = np.asarray(r["rawc"], dtype=np.float64)
        rawf = np.asarray(r["rawf"], dtype=np.float64)

        def softsq(s, fb):
            return np.where(s > TINY, -T * np.log(np.maximum(s, 1e-300)), fb)

        # ---- rows (layout [p, b] with v = 128b + p) ----
        rs = svals[:, 0:64:2] + svals[:, 1:64:2]  # [128, 32]
        rowsoft = softsq(rs, BIGF)
        rowstrip = np.concatenate(
            [softsq(rawf[:, 32:48], BIGF), softsq(rawf[:, 48:64], BIGF)], axis=1
        )
        rowraw = np.concatenate(
            [rawf[:, 0:12], rawf[:, 12:24], rawf[:, 24:32]], axis=1
        )
        row_sq = np.maximum(
            np.minimum(np.minimum(rowsoft, rowstrip), rowraw), 0.0
        )
        row_d = np.sqrt(row_sq)

        # ---- cols ----
        colsum = np.concatenate([csv[32 * c, 0:512] for c in range(4)])
        colsoft = softsq(colsum, CAP)  # w < 2048
        colstrip = np.zeros((P, 4))
        for wb in range(4):
            colstrip[:, wb] = (
                svals[:, 64 + 2 * wb] + svals[:, 64 + 2 * wb + 1]
                + svals[:, 72 + 2 * wb] + svals[:, 72 + 2 * wb + 1]
            )
        colstrip_sq = softsq(colstrip, CAP)  # [p, wb], w = 2048 + 128wb + p
        colraw = np.zeros((P, 12))
        for wb in range(12):
            colraw[:, wb] = np.minimum.reduce(
                [rawc[:, 12 * q + wb] for q in range(3)]
            )
        col_d = np.concatenate([
            np.sqrt(np.maximum(colsoft, 0.0)),
            np.sqrt(np.maximum(colstrip_sq, 0.0)).T.reshape(-1),
            np.sqrt(np.maximum(colraw, 0.0)).T.reshape(-1),
        ])
        vals.append(row_d.mean() + col_d.mean())
    return np.asarray(np.mean(vals), dtype=np.float32)


# revision 4
# speedup vs baseline: 1.0161x; 1.0122x over previous
"""Chamfer loss Trainium2 kernel v2 (data-parallel over batch, 8 NeuronCores).

Per-core architecture ("three-region pinline"), replacing the v1 quadrant
scheme. The 4096x4096 sq-distance matrix is split into three w-regions:

  X  (w <  2048)  x-major SOFT: ACT exp evac (T=0.0015) + free accum_out
                  row-sums; column covers via PE ones-matmul col-SUMS
                  accumulated in one resident PSUM bank (4 x 512 chunks at
                  col-tile positions 0/32/64/96, M=1).
  Y  (w in [2048,2560)) y-major SOFT strip: ACT exp evac + free accum_out
                  col-sums; row covers via DVE bf16 exp-MAX chains over the
                  4 w-blocks (exact selection in exp space) + transposed
                  folds.
  R  (w >= 2560)  y-major RAW: DVE tensor_scalar evac (fp16, min BIG) with
                  fused accum_out col-mins (exact), cross-w-block fp16 min
                  chains for row covers + transposed folds.

Row v = min(softmin_X, strip_expmax_Y, raw_R); col w from its region's
cover. Softmin underflow falls back to the other covers (rows) or an
85T cap (cols). The PE runs 2/3-way row-tiled K=24 matmuls
(tile_position) plus 4-way col-tiled M=1 colsum matmuls; engines pace
the pipeline (~92-100us each by the cost model), PE is ~70us cold.

PSUM budget (8 banks, exact): C colsum [128,512] resident (1 bank) +
S soft tile [128,2048] (4) + B raw tile [128,1536] (3); S and B are
single-buffered with chase-split drains: each drain half's evacuation
overlaps the matmuls refilling the other half.

Host: packs error-compensated 3-way bf16 split gram operands replicated
at partition groups 0/32/64/96 (row-tiling reads lhsT/rhs from the
matching 32-partition group), reassembles rows/cols, averages the 8
per-core losses.
"""

import sys

sys.path.insert(0, "/opt/trn_rl_repo")

from contextlib import ExitStack

import ml_dtypes
import numpy as np

import concourse.bacc as bacc
import concourse.tile as tile
from concourse import mybir
from concourse.bass_utils import run_bass_kernel_spmd

BF16 = ml_dtypes.bfloat16

P = 128
V = 4096
KA = 24
WX = 2048  # x-major soft region width
WY = 512  # y-major soft strip width (4 w-blocks)
T_SOFT = 0.0015
BIG = 1.0e30

_cache = {}

# raw region v-panels
RP_OFF = [0, 1536, 3072]
RP_W = [1536, 1536, 1024]


def _build_nc():
    F32 = mybir.dt.float32
    F16 = mybir.dt.float16
    BF = mybir.dt.bfloat16
    mn = mybir.AluOpType.min
    mx = mybir.AluOpType.max
    X = mybir.AxisListType.X
    AF = mybir.ActivationFunctionType

    nc = bacc.Bacc("TRN2", target_bir_lowering=False)
    ax_d = nc.declare_dram_parameter("ax", [P, V], BF, isOutput=False)
    ay_d = nc.declare_dram_parameter("ay", [P, V], BF, isOutput=False)
    svals_d = nc.declare_dram_parameter("svals", [P, 80], F32, isOutput=True)
    csv_d = nc.declare_dram_parameter("csv", [P, 512], F32, isOutput=True)
    rawc_d = nc.declare_dram_parameter("rawc", [P, 80], F32, isOutput=True)
    rawf_d = nc.declare_dram_parameter("rawf", [P, 80], F32, isOutput=True)

    ESCL = -1.0 / T_SOFT

    with tile.TileContext(nc) as tc, ExitStack() as ctx:
        const = ctx.enter_context(tc.tile_pool(name="const", bufs=1))
        accs = ctx.enter_context(tc.tile_pool(name="accs", bufs=1))
        ecop = ctx.enter_context(tc.tile_pool(name="ecop", bufs=3))
        ycop = ctx.enter_context(tc.tile_pool(name="ycop", bufs=2))
        rcop = ctx.enter_context(tc.tile_pool(name="rcop", bufs=3))
        tpp = ctx.enter_context(tc.tile_pool(name="tpp", bufs=2))

        ax_sb = const.tile([P, V], BF)
        ay_sb = const.tile([P, V], BF)
        ones1 = const.tile([P, 1], BF)
        warmsrc = const.tile([1, 1], F32)
        warm = const.tile([1, 1], F32)
        nc.vector.memset(ones1[:], 1.0)
        nc.vector.memset(warmsrc[:], 1.0)
        # input DMAs first (sync queue -- the scalar queue is blocked ~4us
        # by the exp table load), ordered by first consumption: soft-X b=0
        # needs ay0-3 + ax0; the raw stream needs ay5 + ax0-2 right away.
        # first chunks split small so the first matmuls start ASAP
        nc.sync.dma_start(ay_sb[:, 0:256], ay_d[:, 0:256])
        nc.sync.dma_start(ax_sb[:, 0:256], ax_d[:, 0:256])
        nc.sync.dma_start(ay_sb[:, 2560:2816], ay_d[:, 2560:2816])
        nc.sync.dma_start(ay_sb[:, 256:512], ay_d[:, 256:512])
        nc.sync.dma_start(ax_sb[:, 256:512], ax_d[:, 256:512])
        nc.sync.dma_start(ay_sb[:, 2816:3072], ay_d[:, 2816:3072])
        _order = [("y", 1), ("x", 1), ("y", 2), ("x", 2), ("y", 6), ("y", 3),
                  ("x", 3), ("y", 4), ("y", 7), ("x", 4), ("x", 5), ("x", 6),
                  ("x", 7)]
        for which, k in _order:
            o = 512 * k
            src, dst = (ay_d, ay_sb) if which == "y" else (ax_d, ax_sb)
            nc.sync.dma_start(dst[:, o : o + 512], src[:, o : o + 512])
        nc.scalar.activation(warm[:], warmsrc[:], AF.Exp)

        # accumulators / outputs
        # svals: [0:64] soft-X rowsum halves (2 per v-block);
        #        [64:72] strip colsum (h=0, slot 64+2wb+q);
        #        [72:80] strip colsum (h=1, slot 72+2wb+q)
        svals = accs.tile([P, 80], F32, name="svals")
        csv = accs.tile([P, 512], F32, name="csv")
        # rawc: raw col partials, slot 12q+wb
        rawc = accs.tile([P, 80], F32, name="rawc")
        # rawf: [0:12] raw rows q0, [12:24] q1, [24:32] q2;
        #       [32:48] strip rows q0 (expmax), [48:64] strip rows q1
        rawf = accs.tile([P, 80], F32, name="rawf")
        yracc = accs.tile([P, 2 * 2048], BF, name="yracc")
        rracc = accs.tile([P, V], F16, name="rracc")

        with tc.tile_pool(name="psumC", bufs=1, space="PSUM") as psumC, \
             tc.tile_pool(name="psumS", bufs=2, space="PSUM") as psumS, \
             tc.tile_pool(name="psumB", bufs=1, space="PSUM") as psumB:
            C = psumC.tile([P, 512], F32, name="C")
            B = psumB.tile([P, 1536], F32, name="B")

            # S-stream jobs: soft-X v-blocks with strip jobs mixed in evenly
            xjobs = [("x", b) for b in range(32)]
            yjobs = [("y", (wb, q)) for q in range(2) for wb in range(4)]
            sstream = []
            yi = iter(yjobs)
            for b in range(32):
                sstream.append(xjobs[b])
                if b % 3 == 2:
                    nx = next(yi, None)
                    if nx is not None:
                        sstream.append(nx)
            sstream.extend(yi)
            rjobs = [("r", (wb, q)) for q in range(3) for wb in range(12)]

            # interleave: front-load the B-stream (raw, DVE-paced) so it
            # finishes ~10 soft jobs early -- its transpose+fold epilogue
            # then overlaps ACT's final soft jobs instead of trailing them
            sjobs = []
            ri = 0
            for i, sj in enumerate(sstream):
                want = min(len(rjobs), (i * len(rjobs)) // 29 + 2)
                while ri < want:
                    sjobs.append(rjobs[ri])
                    ri += 1
                sjobs.append(sj)
            while ri < len(rjobs):
                sjobs.append(rjobs[ri])
                ri += 1

            def soft_job(ct, lhs_ap, rhs_base, rhs_off, slots):
                """Two [128,1024] S-pool tiles; all 4 MMs 4-way row-tiled
                (groups 0-3) issued before the two exp evacs."""
                tiles = [psumS.tile([P, 1024], F32, name="S", tag="S")
                         for _ in range(2)]
                for h in range(2):
                    for j in range(2):
                        g = 2 * h + j
                        o = h * 1024 + j * 512
                        nc.tensor.matmul(
                            tiles[h][:, j * 512 : (j + 1) * 512],
                            lhs_ap(g),
                            rhs_base(g)[:, rhs_off + o : rhs_off + o + 512],
                            start=True, stop=True, skip_group_check=True,
                            tile_position=(32 * g, 0),
                        )
                for h in range(2):
                    nc.scalar.activation(
                        ct[:, h * 1024 : (h + 1) * 1024], tiles[h][:],
                        AF.Exp, scale=ESCL,
                        accum_out=svals[:, slots[h] : slots[h] + 1],
                    )

            pending_cs = []  # delayed colsum emissions: (ct, b)

            def emit_pending_colsums(keep):
                while len(pending_cs) > keep:
                    ct, b, first, last = pending_cs.pop(0)
                    for c in range(4):
                        nc.tensor.matmul(
                            C[32 * c : 32 * c + 1, 0:512],
                            ones1[:],
                            ct[:, c * 512 : (c + 1) * 512],
                            start=first, stop=last,
                            skip_group_check=True,
                            tile_position=(0, 32 * c),
                        )
                    if last:
                        nc.scalar.copy(csv[:], C[:])
                        nc.scalar.dma_start(csv_d[:], csv[:])

            nx = 0
            for job, arg in sjobs:
                if job == "x":
                    b = arg
                    ct = ecop.tile([P, 2048], BF, name="ctx", tag="ctx")
                    soft_job(
                        ct,
                        lambda g: ax_sb[32 * g : 32 * g + KA, b * P : (b + 1) * P],
                        lambda g: ay_sb[32 * g : 32 * g + KA, :],
                        0, (2 * b, 2 * b + 1),
                    )
                    pending_cs.append((ct, b, nx == 0, nx == 31))
                    nx += 1
                    emit_pending_colsums(keep=1 if nx < 32 else 0)
                elif job == "y":
                    wb, q = arg
                    if wb == 0:
                        yct = yracc[:, q * 2048 : (q + 1) * 2048]
                    else:
                        ytile = ycop.tile([P, 2048], BF, name="cty", tag="cty")
                        yct = ytile[:]
                    soft_job(
                        yct,
                        lambda g: ay_sb[32 * g : 32 * g + KA,
                                        WX + wb * P : WX + (wb + 1) * P],
                        lambda g: ax_sb[32 * g : 32 * g + KA, :],
                        q * 2048,
                        (64 + 2 * wb + q, 72 + 2 * wb + q),
                    )
                    if wb != 0:
                        nc.vector.tensor_tensor(
                            yracc[:, q * 2048 : (q + 1) * 2048], yct,
                            yracc[:, q * 2048 : (q + 1) * 2048], mx,
                        )
                    if wb == 3:
                        # strip panel complete: transpose + expmax fold
                        tpy = tpp.tile([P, 2048], BF, name="tpy", tag="tpy")
                        nc.sync.dma_start_transpose(
                            tpy[:].rearrange("p (c j) -> p c j", c=16),
                            yracc[:, q * 2048 : (q + 1) * 2048],
                        )
                        t3 = tpy[:].rearrange("p (c j) -> p c j", c=16)
                        nc.vector.tensor_tensor(
                            t3[:, :, 0:64], t3[:, :, 0:64], t3[:, :, 64:128], mx
                        )
                        nc.vector.tensor_tensor(
                            t3[:, :, 0:32], t3[:, :, 0:32], t3[:, :, 32:64], mx
                        )
                        nc.vector.tensor_reduce(
                            rawf[:, 32 + 16 * q : 48 + 16 * q], t3[:, :, 0:32],
                            axis=X, op=mx,
                        )
                        nc.sync.dma_start(
                            rawf_d[:, 32 + 16 * q : 48 + 16 * q],
                            rawf[:, 32 + 16 * q : 48 + 16 * q],
                        )
                else:  # raw
                    wb, q = arg
                    off, w = RP_OFF[q], RP_W[q]
                    for j in range(w // 512):
                        g = (j + 2 * (wb % 2)) % 4
                        nc.tensor.matmul(
                            B[:, j * 512 : (j + 1) * 512],
                            ay_sb[32 * g : 32 * g + KA,
                                  WX + WY + wb * P : WX + WY + (wb + 1) * P],
                            ax_sb[32 * g : 32 * g + KA,
                                  off + j * 512 : off + (j + 1) * 512],
                            start=True, stop=True, skip_group_check=True,
                            tile_position=(32 * g, 0),
                        )
                    if wb == 0:
                        rt = rracc[:, off : off + w]
                    else:
                        rtile = rcop.tile([P, 1536], F16, name="ctr", tag="ctr")
                        rt = rtile[:, 0:w]
                    s0 = 12 * q + wb
                    nc.vector.tensor_scalar(
                        out=rt[:], in0=B[:, 0:w],
                        scalar1=BIG, scalar2=None, op0=mn, op1=mn,
                        accum_out=rawc[:, s0 : s0 + 1],
                    )
                    if wb != 0:
                        nc.vector.tensor_tensor(
                            rracc[:, off : off + w], rt,
                            rracc[:, off : off + w], mn,
                        )
                    if wb == 11:
                        nch = w // P
                        tpr = tpp.tile([P, 1536], F16, name="tpr", tag="tpr")
                        nc.sync.dma_start_transpose(
                            tpr[:, 0:w].rearrange("p (c j) -> p c j", c=nch),
                            rracc[:, off : off + w],
                        )
                        t3 = tpr[:, 0:w].rearrange("p (c j) -> p c j", c=nch)
                        nc.vector.tensor_tensor(
                            t3[:, :, 0:64], t3[:, :, 0:64], t3[:, :, 64:128], mn
                        )
                        nc.vector.tensor_tensor(
                            t3[:, :, 0:32], t3[:, :, 0:32], t3[:, :, 32:64], mn
                        )
                        fs = [0, 12, 24][q]
                        nc.vector.tensor_reduce(
                            rawf[:, fs : fs + nch], t3[:, :, 0:32], axis=X, op=mn
                        )
                        nc.sync.dma_start(
                            rawf_d[:, fs : fs + nch], rawf[:, fs : fs + nch]
                        )

        nc.scalar.dma_start(svals_d[:], svals[:])
        nc.scalar.dma_start(rawc_d[:], rawc[:])

    nc.finalize()
    return nc


def _split3(v):
    f32 = np.float32
    h = v.astype(BF16)
    m = (v - h.astype(f32)).astype(BF16)
    l = (v - h.astype(f32) - m.astype(f32)).astype(BF16)
    return h, m, l


def _augment(x, y):
    """(V,3) fp32 -> AX, AY [24, V] bf16 split gram operands (symmetric)."""
    f32 = np.float32
    yy = (-2.0 * y).astype(f32)
    xh, xm, xl = _split3(x)
    yh, ym, yl = _split3(yy)
    x2 = np.einsum("vc,vc->v", x.astype(np.float64), x.astype(np.float64)).astype(f32)
    y2 = np.einsum("vc,vc->v", y.astype(np.float64), y.astype(np.float64)).astype(f32)
    x2h, x2m, x2l = _split3(x2)
    y2h, y2m, y2l = _split3(y2)
    one = np.ones(V, dtype=BF16)

    def cols(a):
        return [a[:, 0], a[:, 1], a[:, 2]]

    ax = np.stack(
        cols(xh) + cols(xh) + cols(xm) + cols(xh) + cols(xl) + cols(xm)
        + [x2h, x2m, x2l, one, one, one]
    )
    ay = np.stack(
        cols(yh) + cols(ym) + cols(yh) + cols(yl) + cols(yh) + cols(ym)
        + [one, one, one, y2h, y2m, y2l]
    )
    return ax, ay


def _replicate(a):
    out = np.zeros((P, V), dtype=BF16)
    for g in range(4):
        out[32 * g : 32 * g + KA] = a
    return out


def kernel(x, y):
    x = np.asarray(x, dtype=np.float32)
    y = np.asarray(y, dtype=np.float32)
    n = x.shape[0]
    assert x.shape == (n, V, 3) and y.shape == (n, V, 3) and n == 8

    if "nc" not in _cache:
        _cache["nc"] = _build_nc()
    nc = _cache["nc"]

    in_maps = []
    for i in range(n):
        ax, ay = _augment(x[i], y[i])
        in_maps.append({"ax": _replicate(ax), "ay": _replicate(ay)})

    res = run_bass_kernel_spmd(
        nc, in_maps, list(range(n)), trace=_cache.get("trace", False)
    )
    _cache["last"] = res

    T = T_SOFT
    CAP = 85.0 * T
    BIGF = 1.0e30
    TINY = 1e-35
    vals = []
    for i in range(n):
        r = res.results[i]
        svals = np.asarray(r["svals"], dtype=np.float64)
        csv = np.asarray(r["csv"], dtype=np.float64)
        rawc = np.asarray(r["rawc"], dtype=np.float64)
        rawf = np.asarray(r["rawf"], dtype=np.float64)

        def softsq(s, fb):
            return np.where(s > TINY, -T * np.log(np.maximum(s, 1e-300)), fb)

        # ---- rows (layout [p, b] with v = 128b + p) ----
        rs = svals[:, 0:64:2] + svals[:, 1:64:2]  # [128, 32]
        rowsoft = softsq(rs, BIGF)
        rowstrip = np.concatenate(
            [softsq(rawf[:, 32:48], BIGF), softsq(rawf[:, 48:64], BIGF)], axis=1
        )
        rowraw = np.concatenate(
            [rawf[:, 0:12], rawf[:, 12:24], rawf[:, 24:32]], axis=1
        )
        row_sq = np.maximum(
            np.minimum(np.minimum(rowsoft, rowstrip), rowraw), 0.0
        )
        row_d = np.sqrt(row_sq)

        # ---- cols ----
        colsum = np.concatenate([csv[32 * c, 0:512] for c in range(4)])
        colsoft = softsq(colsum, CAP)  # w < 2048
        colstrip = np.zeros((P, 4))
        for wb in range(4):
            colstrip[:, wb] = (
                svals[:, 64 + 2 * wb] + svals[:, 64 + 2 * wb + 1]
                + svals[:, 72 + 2 * wb] + svals[:, 72 + 2 * wb + 1]
            )
        colstrip_sq = softsq(colstrip, CAP)  # [p, wb], w = 2048 + 128wb + p
        colraw = np.zeros((P, 12))
        for wb in range(12):
            colraw[:, wb] = np.minimum.reduce(
                [rawc[:, 12 * q + wb] for q in range(3)]
            )
        col_d = np.concatenate([
            np.sqrt(np.maximum(colsoft, 0.0)),
            np.sqrt(np.maximum(colstrip_sq, 0.0)).T.reshape(-1),
            np.sqrt(np.maximum(colraw, 0.0)).T.reshape(-1),
        ])
        vals.append(row_d.mean() + col_d.mean())
    return np.asarray(np.mean(vals), dtype=np.float32)
